# revision 51
# baseline (speedup 1.0000x reference)
"""Multi-head attention (B=4, S=2048, D=1024, H=16) on 8 Trainium2 NeuronCores.

Sharding: data-parallel over 4 batches x tensor-parallel over head halves
(2 groups of 8 heads).  core c -> batch c//2, heads (c%2)*8 .. (c%2)*8+7.
Host sums the two partial outputs per batch and de-transposes.

v3 design (vs v1 fp16 baseline, 425us):
  - QKV projection in fp8e4m3 DoubleRow matmuls with host-side hi/lo error
    feedback: q = 2^-5 * (xh.wh + xh.wl + xl.wh); w pre-scaled by 32 on the
    host so its values sit in fp8-normal range, residuals land in
    subnormals (their 2.3% relative error is on a 2.4%-magnitude term ->
    ~0.1% total).  12 DR k-tile pairs replace 8 fp16 matmuls per tile:
    projection PE time 82us -> 61.5us, numeric error ~0.2%.
  - attention core stays fp16 (fp8 scores/expm would inject ~2.4% -> over
    the 2e-2 budget): QK + AV per (pair, 512-q-block, 128-k-chunk) with
    exp on ACT ([128,2,512] tiles, 2 PSUM banks, double buffered) and the
    mask multiply on DVE fp16 2x.
  - out-projection computed TRANSPOSED (outT[d,s]) so the bias is
    per-partition: PSUM->SBUF copy + bias is one DVE tensor_scalar (gpsimd
    cannot read PSUM), and the 32 K=1 bias matmuls of v1 die.
  - denominator+ctx copy-out merged into one [65,512] DVE copy per head
    into a `cu` tile; reciprocal writes partition-shifted into a [1,8,512]
    row (shifted-base engine APs verified on HW); normalize multiplies run
    on Pool into ctxN (pair layout), which feeds the out-proj as lhsT.
  - mask streamed per q-block ([128,16,512] ring) instead of held whole.
  - one flat (qb, c, j) software pipeline; k/v/q-projection tiles and
    out-projection blocks are emitted as "fillers" sorted by need-time so
    the PE queue never head-of-line blocks: the projection of q-block n+1
    and the out-projection of q-block n-1 hide in the attention PE slack
    of q-block n.

Cost-model engine budgets per core: PE ~313us busy (QK 109 + AV 109 +
proj 61.5 + out 27 + p-states), ACT ~267us (256 exps), DVE ~244us,
Pool ~92us.  Total 357.7us (PE-bound, ~88% PE occupancy).
"""

import os
import sys
import math
from contextlib import ExitStack

import numpy as np
import ml_dtypes

if "/opt/trn_rl_repo" not in sys.path:
    sys.path.insert(0, "/opt/trn_rl_repo")

B, S, D, H = 4, 2048, 1024, 16
DH = 64          # head dim
HPC = 8          # heads per core
CD = HPC * DH    # 512 cols per core per q/k/v
NCORES = 8

SKC = 16         # s_k chunks of 128
QB = 4           # q blocks of 512
NPAIR = 4        # head pairs per core
WP = 4           # contraction pair-blocks of 256 (d = pr*256 + i*128 + p)
WSCALE = 2.0 ** -5   # w pre-scaled by 32 on host

E4M3 = ml_dtypes.float8_e4m3


def _build():
    import concourse.bass as bass
    import concourse.mybir as mybir
    import concourse.tile as tile
    from concourse import bacc
    from concourse.bass import ds, ts

    f32 = mybir.dt.float32
    f16 = mybir.dt.float16
    f8 = mybir.dt.float8e4
    Alu = mybir.AluOpType
    Act = mybir.ActivationFunctionType
    DR = mybir.MatmulPerfMode.DoubleRow

    nc = bacc.Bacc(name="mha8v3")

    # x layout [p, nb, pr, i, 512]: each 512-s block is contiguous per
    # partition, so its DMA is 128 descriptors instead of 1024
    xh_d = nc.dram_tensor("xh", [128, 4, WP, 2, 512], f8, kind="ExternalInput")
    xl_d = nc.dram_tensor("xl", [128, 4, WP, 2, 512], f8, kind="ExternalInput")
    # wq/wk m-major [p, m, pr, i, 128] so the m=0 slices (all the first
    # projection tiles need) are 1KB-contiguous DMAs; wv stays pr-major
    w_d = {}
    for n in ("wqh", "wql", "wkh", "wkl"):
        w_d[n] = nc.dram_tensor(n, [128, 4, WP, 2, 128], f8, kind="ExternalInput")
    for n in ("wvh", "wvl"):
        w_d[n] = nc.dram_tensor(n, [128, WP, 2, CD], f8, kind="ExternalInput")
    bq_d = nc.dram_tensor("bq", [CD], f32, kind="ExternalInput")
    bk_d = nc.dram_tensor("bk", [CD], f32, kind="ExternalInput")
    bv_d = nc.dram_tensor("bv", [CD], f32, kind="ExternalInput")
    wo_d = nc.dram_tensor("wo", [CD, D], f16, kind="ExternalInput")
    bo_d = nc.dram_tensor("bo", [D], f32, kind="ExternalInput")
    mask_d = nc.dram_tensor("mask", [S, S], f16, kind="ExternalInput")
    out_d = nc.dram_tensor("out", [D, S], f16, kind="ExternalOutput")

    with tile.TileContext(nc) as tc, ExitStack() as top:
        const = top.enter_context(tc.tile_pool(name="const", bufs=1))

        exp_bias = const.tile([128, 1], f32)
        nc.vector.memset(exp_bias, -4.0)  # exp(s/8 - 4): keeps fp16 expm in range
        act_warm = const.tile([128, 1], f32)
        nc.scalar.activation(out=act_warm, in_=exp_bias, func=Act.Exp, scale=1.0)

        # biases for q/k: [128, 4] -> column m holds b[m*128+p]
        bq_sb = const.tile([128, NPAIR], f32)
        bk_sb = const.tile([128, NPAIR], f32)
        bo_col = const.tile([128, 8], f32)

        def dma_biases():
            nc.sync.dma_start(out=bq_sb, in_=bq_d.rearrange("(m p) -> p m", p=128))
            nc.sync.dma_start(out=bk_sb, in_=bk_d.rearrange("(m p) -> p m", p=128))
            nc.sync.dma_start(out=bo_col, in_=bo_d.rearrange("(n p) -> p n", p=128))
        wo_sb = const.tile([128, 4, D], f16)

        def dma_wo():
            nc.sync.dma_start(
                out=wo_sb, in_=wo_d.rearrange("(r p) n -> p r n", p=128)
            )

        bv_row = const.tile([1, CD], f32)
        bv_bc = const.tile([128, CD], f32)

        def dma_bv():
            nc.sync.dma_start(out=bv_row, in_=bv_d[None, :])
            nc.gpsimd.partition_broadcast(bv_bc, bv_row)

        # fp8 activations + weights (persistent; x stays live for late q-proj)
        x_pool = top.enter_context(tc.tile_pool(name="xp", bufs=1))
        xh_sb = x_pool.tile([128, 4, WP, 2, 512], f8)
        xl_sb = x_pool.tile([128, 4, WP, 2, 512], f8)
        w_pool = top.enter_context(tc.tile_pool(name="wp", bufs=1))
        w_sb = {}
        for n in ("wqh", "wql", "wkh", "wkl"):
            w_sb[n] = w_pool.tile([128, 4, WP, 2, 128], f8, name=n)
        for n in ("wvh", "wvl"):
            w_sb[n] = w_pool.tile([128, WP, 2, CD], f8, name=n)

        # DMA transfers serialize on the model's DMA-engines track in gen
        # order, so the lead minimizes critical BYTES and gen slots: x0 and
        # the 1KB m0 slices of wk/wq land first (first exp ~7us)
        # per-queue FIFOs, globally paced against the serialized DMA track:
        # sync: x blocks + biases; gpsimd: wk/wv; scalar: wq + mask qb0
        nc.sync.dma_start(out=xh_sb[:, 0], in_=xh_d[:, 0, :, :, :])
        nc.gpsimd.dma_start(out=w_sb["wkh"][:, 0], in_=w_d["wkh"][:, 0])
        nc.scalar.dma_start(out=w_sb["wqh"][:, 0], in_=w_d["wqh"][:, 0])
        nc.gpsimd.dma_start(out=w_sb["wvh"], in_=w_d["wvh"][:, :, :, :])
        nc.sync.dma_start(out=xl_sb[:, 0], in_=xl_d[:, 0, :, :, :])
        nc.gpsimd.dma_start(out=w_sb["wkl"][:, 0], in_=w_d["wkl"][:, 0])
        nc.scalar.dma_start(out=w_sb["wql"][:, 0], in_=w_d["wql"][:, 0])
        nc.gpsimd.dma_start(out=w_sb["wvl"], in_=w_d["wvl"][:, :, :, :])
        dma_biases()
        dma_bv()
        nc.gpsimd.dma_start(out=w_sb["wkh"][:, 1:4], in_=w_d["wkh"][:, 1:4])
        nc.gpsimd.dma_start(out=w_sb["wkl"][:, 1:4], in_=w_d["wkl"][:, 1:4])
        for nb in (1, 2, 3):
            for t_d, t_sb in ((xh_d, xh_sb), (xl_d, xl_sb)):
                nc.sync.dma_start(out=t_sb[:, nb], in_=t_d[:, nb, :, :, :])

        # persistent activations
        qk_pool = top.enter_context(tc.tile_pool(name="qk", bufs=1))
        kT_sb = qk_pool.tile([128, NPAIR, S], f16)   # pair m: p<64 head 2m, else 2m+1
        qt_pool = top.enter_context(tc.tile_pool(name="qtp", bufs=2))
        qtile = {}   # qb -> [128, NPAIR, 512] q slab ring
        v_pool = top.enter_context(tc.tile_pool(name="vp", bufs=1))
        v_sb = v_pool.tile([128, SKC, HPC, 66], f16)  # [..,0:64]=v, 64=ones
        nc.gpsimd.memset(v_sb[:, :, :, 64:65], 1.0)

        # rings
        mask_pool = top.enter_context(tc.tile_pool(name="mkp", bufs=2))
        cu_pool = top.enter_context(tc.tile_pool(name="cup", bufs=2))
        ctxn_pool = top.enter_context(tc.tile_pool(name="cnp", bufs=2))
        rc_pool = top.enter_context(tc.tile_pool(name="rcp", bufs=2))
        expm_pool = top.enter_context(tc.tile_pool(name="exp", bufs=5))
        ostg_pool = top.enter_context(tc.tile_pool(name="ostg", bufs=4))

        sc_ps = top.enter_context(tc.tile_pool(name="scps", bufs=2, space="PSUM"))
        ctx_ps = top.enter_context(tc.tile_pool(name="ctxps", bufs=2, space="PSUM"))
        gp_ps = top.enter_context(tc.tile_pool(name="gps", bufs=2, space="PSUM"))

        mk = {}      # qb -> mask tile
        cu = {}      # qb -> merged ctx+den tile [65, 8, 512]
        ctxn = {}    # qb -> normalized ctx (pair layout)
        ctx_t = {}   # (qb, c) -> [ctx psum a=0, a=1]

        def dma_mask(qb, eng=None):
            mkt = mask_pool.tile([128, SKC, 512], f16, tag="mk", name=f"mk{qb}")
            mk[qb] = mkt
            for j in range(SKC):
                (eng or nc.sync).dma_start(
                    out=mkt[:, j, :],
                    in_=mask_d[ds(j * 128, 128), ds(qb * 512, 512)],
                )

        # ---------------- fp8 DR projection tiles -------------------------
        def dr_seq(wh, wl, swap):
            # 12 (lhsT, rhs) k-tile pairs: (wh,xh)x4, (wl,xh)x4, (wh,xl)x4
            seq = []
            for wsb, xsb in ((wh, xh_sb), (wl, xh_sb), (wh, xl_sb)):
                for pr in range(WP):
                    seq.append((xsb, wsb, pr) if swap else (wsb, xsb, pr))
            return seq

        def proj_qk(which, m, nb):
            w_h, w_l = w_sb[f"w{which}h"], w_sb[f"w{which}l"]
            b_sb = bq_sb if which == "q" else bk_sb
            if which == "q":
                if m == 0:
                    qtile[nb] = qt_pool.tile(
                        [128, NPAIR, 512], f16, tag="qt", name=f"qt{nb}"
                    )
                dst = qtile[nb][:, m, :]
            else:
                dst = kT_sb[:, m, ds(nb * 512, 512)]
            pps = gp_ps.tile([128, 512], f32, tag="gps", name=f"p{which}{m}{nb}")
            for t, (lt, rt, pr) in enumerate(dr_seq(w_h, w_l, swap=False)):
                nc.tensor.matmul(
                    pps,
                    lhsT=lt[:, m, pr, :, :],
                    rhs=rt[:, nb, pr, :, :],
                    perf_mode=DR,
                    start=(t == 0),
                    stop=(t == 11),
                )
            nc.vector.tensor_scalar(
                out=dst,
                in0=pps,
                scalar1=WSCALE,
                scalar2=b_sb[:, ds(m, 1)],
                op0=Alu.mult,
                op1=Alu.add,
            )

        def proj_v(m16):
            pps = gp_ps.tile([128, 512], f32, tag="gps", name=f"pv{m16}")
            for t, (lt, rt, pr) in enumerate(
                dr_seq(w_sb["wvh"], w_sb["wvl"], swap=True)
            ):
                nc.tensor.matmul(
                    pps,
                    lhsT=lt[:, m16 // 4, pr, :, ds((m16 % 4) * 128, 128)],
                    rhs=rt[:, pr, :, :],
                    perf_mode=DR,
                    start=(t == 0),
                    stop=(t == 11),
                )
            nc.vector.scalar_tensor_tensor(
                out=v_sb[:, m16, :, 0:64],
                in0=pps.rearrange("p (h e) -> p h e", h=HPC),
                scalar=WSCALE,
                in1=bv_bc.rearrange("p (h e) -> p h e", h=HPC),
                op0=Alu.mult,
                op1=Alu.add,
            )

        # ---------------- attention stages ---------------------------------
        def emit_qk(qb, c, j):
            sc = sc_ps.tile([128, 2, 512], f32, tag="scps", name="sc")
            for a in range(2):
                nc.tensor.matmul(
                    sc[:, a, :],
                    lhsT=kT_sb[ds(a * 64, 64), c, ds(j * 128, 128)],
                    rhs=qtile[qb][ds(a * 64, 64), c, :],
                    start=True,
                    stop=True,
                )
            return sc

        def emit_mask_av(qb, c, j, sc):
            expm = expm_pool.tile([128, 2, 512], f16, tag="expm")
            nc.scalar.activation(
                out=expm, in_=sc, func=Act.Exp,
                scale=1.0 / math.sqrt(DH), bias=exp_bias,
            )
            expm2 = expm_pool.tile([128, 2, 512], f16, tag="expm2")
            nc.vector.tensor_tensor(
                out=expm2,
                in0=expm,
                in1=mk[qb][:, j, None, :].to_broadcast((128, 2, 512)),
                op=Alu.mult,
            )
            for a in range(2):
                nc.tensor.matmul(
                    ctx_t[(qb, c)][a][0:65, :],
                    lhsT=v_sb[:, j, c * 2 + a, 0:65],
                    rhs=expm2[:, a, :],
                    start=(j == 0),
                    stop=(j == SKC - 1),
                )

        def alloc_ctx(qb, c):
            ctx_t[(qb, c)] = [
                ctx_ps.tile([128, 512], f32, tag=f"ctx{a}", bufs=1, name=f"ctx{a}")
                for a in range(2)
            ]
            if c == 0:
                cu[qb] = cu_pool.tile(
                    [65, HPC, 512], f16, tag="cu", name=f"cu{qb}"
                )

        def emit_ctx_out(qb, c):
            for a in range(2):
                nc.vector.tensor_copy(
                    out=cu[qb][0:65, c * 2 + a, :],
                    in_=ctx_t[(qb, c)][a][0:65, :],
                )

        def emit_norm_pair(qb, c):
            # incremental per-pair normalize right after the pair's ctx
            # copy-out: only pair 3's chain remains in the tail
            if c == 0:
                ctxn[qb] = ctxn_pool.tile(
                    [128, NPAIR, 512], f16, tag="ctxn", name=f"cn{qb}"
                )
            # reciprocal of den rows (partition 64 -> 0 shifted)
            dengh = rc_pool.tile(
                [1, 2, 512], f16, tag="dengh", bufs=2, name=f"dgh{qb}{c}"
            )
            with nc.allow_low_precision(reason="1/denom feeds fp16 normalize"):
                nc.vector.reciprocal(dengh, cu[qb][64:65, ds(2 * c, 2), :])
            for a in range(2):
                h = 2 * c + a
                rbc = rc_pool.tile(
                    [128, 512], f16, tag="rbc", bufs=3, name=f"rbc{h}"
                )
                nc.gpsimd.partition_broadcast(rbc, dengh[0:1, a, :])
                # both SBUF inputs must share a base partition (walrus
                # IBIR297); rbc rows are identical, so always read 0:64.
                nc.gpsimd.tensor_tensor(
                    out=ctxn[qb][ds(a * 64, 64), c, :],
                    in0=cu[qb][0:64, h, :],
                    in1=rbc[0:64, :],
                    op=Alu.mult,
                )

        def emit_tail_norm(qb, c):
            # final pair: its psum banks are never reused, so skip the cu
            # copy entirely -- reciprocal reads the den row and the
            # normalize TT reads ctxU straight from PSUM (PSUM+SBUF operand
            # mix is exempt from IBIR297).  Cuts ~2us off the drain chain.
            dengh = rc_pool.tile([1, 2, 512], f16, tag="dengh", bufs=2, name="dghT")
            with nc.allow_low_precision(reason="1/denom feeds fp16 normalize"):
                for a in range(2):
                    nc.vector.reciprocal(
                        dengh[0:1, ds(a, 1), :], ctx_t[(qb, c)][a][64:65, :]
                    )
            for a in range(2):
                rbc = rc_pool.tile([128, 512], f16, tag="rbc", bufs=3, name="rbcT")
                nc.gpsimd.partition_broadcast(rbc, dengh[0:1, a, :])
                nc.vector.tensor_tensor(
                    out=ctxn[qb][ds(a * 64, 64), c, :],
                    in0=ctx_t[(qb, c)][a][0:64, :],
                    in1=rbc[0:64, :],
                    op=Alu.mult,
                )

        def emit_outproj(qb, db):
            ops = gp_ps.tile([128, 512], f32, tag="gps", name=f"op{qb}{db}")
            for r in range(4):
                nc.tensor.matmul(
                    ops,
                    lhsT=wo_sb[:, r, ds(db * 128, 128)],
                    rhs=ctxn[qb][:, r, :],
                    start=(r == 0),
                    stop=(r == 3),
                )
            ost = ostg_pool.tile([128, 512], f16, tag="ostg", name="ost")
            # gpsimd cannot read PSUM (walrus) -> DVE for the copy+bias
            nc.vector.tensor_scalar_add(out=ost, in0=ops, scalar1=bo_col[:, ds(db, 1)])
            # scalar-engine DGE queue: keeps out DMAs off the sync queue,
            # where a blocked mask-prefetch would head-of-line block them
            nc.scalar.dma_start(
                out=out_d[ds(db * 128, 128), ds(qb * 512, 512)], in_=ost
            )

        # ---------------- schedule -----------------------------------------
        # fillers: (due_iter, kind, fn). Emitted when the flat iteration
        # index reaches due_iter; PE fillers are ordered by true need-time
        # so the in-order PE queue never blocks the attention stream.
        mkt0 = mask_pool.tile([128, SKC, 512], f16, tag="mk", name="mk0")
        mk[0] = mkt0
        for j in range(10):
            nc.scalar.dma_start(
                out=mkt0[:, j, :], in_=mask_d[ds(j * 128, 128), 0:512]
            )
        nc.scalar.dma_start(out=w_sb["wqh"][:, 1:4], in_=w_d["wqh"][:, 1:4])
        nc.scalar.dma_start(out=w_sb["wql"][:, 1:4], in_=w_d["wql"][:, 1:4])
        for j in range(10, SKC):
            nc.scalar.dma_start(
                out=mkt0[:, j, :], in_=mask_d[ds(j * 128, 128), 0:512]
            )
        dma_mask(1)             # sync queue tail: lands ~25us, needed ~85us

        fillers = []

        def it_of(qb, c, j):
            return qb * 64 + c * 16 + j

        LEAD = 6  # emit fillers this many iterations before first use
        for m in range(NPAIR):          # k tiles: needed at (0, m, 4*nb)
            for nb in range(4):
                if m == 0 and nb == 0:
                    continue  # emitted in the preamble
                due = it_of(0, m, 4 * nb) - LEAD
                if m == 0 and nb == 1:
                    due = 1   # xh1 lands ~11us; don't emit earlier
                fillers.append((due, proj_qk, ("k", m, nb)))
        for m in range(1, NPAIR):       # q0 m1-3: wq m-slices land ~17us
            fillers.append((11 + m, proj_qk, ("q", m, 0)))
        for m16 in range(SKC):          # v tiles: needed at (0, 0, m16)
            if m16 < 2:
                continue  # preamble
            fillers.append((max(it_of(0, 0, m16) - 4, m16 - 8), proj_v, (m16,)))
        for qb in range(1, QB):         # q tiles for qb: needed at (qb, m, 0)
            for m in range(NPAIR):
                fillers.append((it_of(qb, m, 0) - 24, proj_qk, ("q", m, qb)))
        for qb in range(QB):            # out-proj: well after normalize(qb)
            for db in range(8):
                fillers.append((qb * 64 + 64 + 16 + db, emit_outproj, (qb, db)))
        for qb in range(2, QB):         # mask block qb loads two qbs ahead
            fillers.append(((qb - 2) * 64 + 8, dma_mask, (qb,)))
        fillers.sort(key=lambda f: f[0])

        # preamble: only tiles whose DMAs land in the first ~8us; anything
        # more would head-of-line block the in-order PE queue
        proj_qk("k", 0, 0)
        proj_qk("q", 0, 0)
        proj_v(0)
        proj_v(1)
        dma_wo()

        all_iters = [
            (qb, c, j) for qb in range(QB) for c in range(NPAIR) for j in range(SKC)
        ]
        pending = None
        deferred_ctx = []   # (qb, c, due_iter)
        deferred_norm = []  # (qb, c, due_iter)
        fi = 0
        for it, (qb, c, j) in enumerate(all_iters):
            if j == 0:
                alloc_ctx(qb, c)
            sc = emit_qk(qb, c, j)
            while fi < len(fillers) and fillers[fi][0] <= it:
                _, fn, args = fillers[fi]
                fn(*args)
                fi += 1
            if pending is not None:
                emit_mask_av(*pending)
                pqb, pc, pj = pending[:3]
                if pj == SKC - 1:
                    deferred_ctx.append((pqb, pc, it + 1))
                    deferred_norm.append((pqb, pc, it + 5))
            while deferred_ctx and deferred_ctx[0][2] <= it:
                dqb, dc, _ = deferred_ctx.pop(0)
                emit_ctx_out(dqb, dc)
            while deferred_norm and deferred_norm[0][2] <= it:
                dqb, dc, _ = deferred_norm.pop(0)
                emit_norm_pair(dqb, dc)
            pending = (qb, c, j, sc)
        emit_mask_av(*pending)
        for dqb, dc, _ in deferred_ctx:
            emit_ctx_out(dqb, dc)
        for dqb, dc, _ in deferred_norm:
            emit_norm_pair(dqb, dc)
        emit_tail_norm(pending[0], pending[1])
        while fi < len(fillers):
            _, fn, args = fillers[fi]
            fn(*args)
            fi += 1

    nc.compile()
    return nc


_NC = None


def _get_nc():
    global _NC
    if _NC is None:
        _NC = _build()
    return _NC


def _quant_pair(a32, kind):
    """a32 [1024, N] f32 -> (hi, lo) e4m3 in device layout.
    kind: "x" -> [p, nb, pr, i, 512]; "qk" -> [p, m, pr, i, 128];
    "v" -> [p, pr, i, 512].  (d = pr*256 + i*128 + p everywhere)"""
    hi = a32.astype(E4M3)
    lo = (a32 - hi.astype(np.float32)).astype(E4M3)

    def lay(t):
        if kind == "x":
            return np.ascontiguousarray(
                t.reshape(WP, 2, 128, 4, 512).transpose(2, 3, 0, 1, 4)
            )
        if kind == "qk":
            return np.ascontiguousarray(
                t.reshape(WP, 2, 128, 4, 128).transpose(2, 3, 0, 1, 4)
            )
        return np.ascontiguousarray(
            t.reshape(WP, 2, 128, CD).transpose(2, 0, 1, 3)
        )

    return lay(hi), lay(lo)


def make_in_maps(inputs):
    x = np.asarray(inputs["x"], dtype=np.float32)
    mask = np.asarray(inputs["mask"], dtype=np.int32)
    w_qkv = np.asarray(inputs["w_qkv"], dtype=np.float32)
    b_qkv = np.asarray(inputs["b_qkv"], dtype=np.float32)
    w_out = np.asarray(inputs["w_out"], dtype=np.float32)
    b_out = np.asarray(inputs["b_out"], dtype=np.float32)

    xhl = [_quant_pair(np.ascontiguousarray(x[b].T), "x") for b in range(B)]
    maskT16 = [
        np.ascontiguousarray(mask[b, 0].T.astype(np.float16)) for b in range(B)
    ]

    in_maps = []
    for core in range(NCORES):
        b = core // 2
        h0 = (core % 2) * CD
        m = {}
        m["xh"], m["xl"] = xhl[b]
        for which, off in (("q", 0), ("k", D), ("v", 2 * D)):
            wh, wl = _quant_pair(
                np.ascontiguousarray(w_qkv[:, off + h0 : off + h0 + CD]) * 32.0,
                "qk" if which in ("q", "k") else "v",
            )
            m[f"w{which}h"], m[f"w{which}l"] = wh, wl
            m[f"b{which}"] = np.ascontiguousarray(b_qkv[off + h0 : off + h0 + CD])
        m["wo"] = np.ascontiguousarray(w_out[h0 : h0 + CD, :]).astype(np.float16)
        m["bo"] = b_out if core % 2 == 0 else np.zeros_like(b_out)
        m["mask"] = maskT16[b]
        in_maps.append(m)
    return in_maps


def gather_out(core_outs):
    # core outputs are transposed partials [D, S]
    return np.stack(
        [
            (core_outs[2 * b].astype(np.float32) + core_outs[2 * b + 1]).T
            for b in range(B)
        ],
        axis=0,
    )


def run(inputs, trace=False):
    """Returns (output, BassKernelResults)."""
    from concourse import bass_utils

    nc = _get_nc()
    in_maps = make_in_maps(inputs)
    res = bass_utils.run_bass_kernel_spmd(
        nc, in_maps, core_ids=list(range(NCORES)), trace=trace
    )
    out = gather_out([r["out"] for r in res.results])
    return out, res


def kernel(**inputs) -> np.ndarray:
    out, _ = run(inputs, trace=False)
    return out
